# revision 15
# baseline (speedup 1.0000x reference)
"""Trainium2 Bass kernel for nn_Head_44203803411019.

Single attention head, B=16 T=2048 C=768 HS=64, fp32 I/O, with the source
quirks: scores scaled by 1/sqrt(C) (not head size) and softmax over the
QUERY axis (axis=1), i.e. a column softmax of the causal-masked score
matrix.

Math: with P = exp(S_masked) (no max-subtraction needed: |S*scl| <= ~2)
and c[k] = sum_q P[q,k], the output is
    out = W @ V = P @ (V / c[:, None])
so the big TxT matrix is never normalized; V rows are scaled instead.

Compute is bf16 on the PE (fp32 PSUM accumulation): ~2.2x the fp32 rate,
fast weight loads, half the SBUF/DMA bytes.  Host-side prep feeds the
device xT (c-major) and [Wq|Wk] packed, so no on-device transposes of x
and Q+K come out of one moving pass.

Per example:
  qkT = [Wq|Wk].T @ xT          [128, T]  rows 0:64 = QT, 64:128 = KT
  vT  = Wv.T @ xT               [64, T] -> transpose -> vn [t,64] blocks
  ST[k,q] = KT_kb.T @ QT        per 128-row k-block (valid q >= kb*128)
  PT = exp(ST/sqrt(C)), diagonal block's lower triangle zeroed (gpsimd)
  c[k] per row via DVE reduces; vnr = vn * (1/c)
  outT[h,q] = sum_kb vnr[kb].T @ PT[kb]; transpose -> natural, DMA out.

Sharding: data-parallel over batch, 2 examples per core, weights
replicated.  Inputs are the FULL tensors; sharding happens here.
"""

import math

import ml_dtypes
import numpy as np

import concourse.bass as bass
import concourse.mybir as mybir
from concourse.bass_utils import run_bass_kernel_spmd
from concourse.masks import make_identity
from concourse.tile import TileContext
from concourse.vector_clock import ScopedClock, VectorClock

try:
    from concourse.tile_sem_assignment import N_PROCS
except ImportError:  # pragma: no cover
    N_PROCS = 27

B, T, C, HS = 16, 2048, 768, 64
NCORES = 8
BPC = B // NCORES          # examples per core
P = 128                    # SBUF partitions
NT = T // P                # 16 t-blocks
NCB = C // P               # 6 c-blocks
QG = 512                   # q-chunk width (PSUM bank)
NQG = T // QG              # 4 q-chunks
SCL = 1.0 / math.sqrt(C)

F32 = mybir.dt.float32
BF16 = mybir.dt.bfloat16

# column offset of k-block kb inside the packed triangular PT buffer
PTOFF = [0] * (NT + 1)
for _kb in range(NT):
    PTOFF[_kb + 1] = PTOFF[_kb] + (T - _kb * P)
PTW = PTOFF[NT]            # 17408 columns total


class _SplitDrainTileContext(TileContext):
    """TileContext whose tail drain splits its sem waits across several
    drain instructions: this neuronxcc build caps sync-wait commands per
    CTRL instruction and rejects the stock single drain-with-N-waits."""

    def _drain_and_barrier(self, tick_clock, wait_clock):
        gc = tick_clock.global_clock
        for p in range(N_PROCS):
            if gc[p] <= 0:
                continue
            partial = VectorClock(
                [gc[q] if q == p else 0 for q in range(N_PROCS)]
            )
            drain_inst = self.nc.sync.drain()
            wait_clock.add_sem_waits(
                drain_inst.ins, ScopedClock({None: partial})
            )
        self.nc.all_engine_barrier()
        popped = self.nc._tile_sem_poison_stack.pop()
        assert popped is self._sem_poison
        self.nc.clear_and_free_semaphores(list(self.sems.allocated().values()))
        self.nc.all_engine_barrier()


def _split_sync_waits(nc, maxw=1):
    """This neuronxcc build rejects >1 sync-wait command on several
    instruction structs (CTRL drains, matmul LDW).  Move excess waits onto
    dedicated same-engine NOPs placed right before the instruction."""
    k = 0
    for f in nc.m.functions:
        for bb in f.blocks:
            new = []
            for inst in bb.instructions:
                si = inst.sync_info
                waits = list(si.on_wait) if si is not None and si.on_wait else []
                if len(waits) > maxw:
                    extra, keep = waits[:-maxw], waits[-maxw:]
                    for i in range(0, len(extra), maxw):
                        k += 1
                        new.append(
                            mybir.InstNoOp(
                                name=f"{inst.name}_sw{k}",
                                engine=inst.engine,
                                bass_nofuse=True,
                                sync_info=mybir.SyncInfo(
                                    on_wait=extra[i:i + maxw], on_update=[]
                                ),
                            )
                        )
                    si.on_wait = keep
                new.append(inst)
            bb.instructions[:] = new


def _build_nc(reps: int = 1) -> bass.Bass:
    nc = bass.Bass()
    xt_in = nc.declare_dram_parameter("xt", [BPC, C, T], BF16, isOutput=False)
    wqk_in = nc.declare_dram_parameter("wqk", [C, P], BF16, isOutput=False)
    wv_in = nc.declare_dram_parameter("wv", [C, HS], BF16, isOutput=False)
    y_out = nc.declare_dram_parameter("out", [BPC, T, HS], F32, isOutput=True)

    with _SplitDrainTileContext(nc) as tc:
        with (
            tc.tile_pool(name="singles", bufs=1) as singles,
            tc.tile_pool(name="xt", bufs=2) as p_xt,
            tc.tile_pool(name="qkv", bufs=2) as p_qkv,
            tc.tile_pool(name="pt", bufs=2) as p_pt,
            tc.tile_pool(name="small", bufs=3) as p_small,
            tc.tile_pool(name="stats", bufs=2) as p_stats,
            tc.tile_pool(name="psS", bufs=2, space="PSUM") as p_psS,
            tc.tile_pool(name="psP", bufs=2, space="PSUM") as p_psP,
            tc.tile_pool(name="psO", bufs=1, space="PSUM") as p_psO,
            tc.tile_pool(name="psT", bufs=1, space="PSUM") as p_psT,
        ):
            identf = singles.tile([HS, HS], F32)
            make_identity(nc, identf)
            ident = singles.tile([HS, HS], BF16, tag="identb")
            nc.vector.tensor_copy(ident, identf)

            # diag mask: 0 where q-col >= k-row, large-negative below
            dmask = singles.tile([P, P], F32)
            nc.gpsimd.memset(dmask, 0.0)
            nc.gpsimd.affine_select(
                out=dmask,
                in_=dmask,
                compare_op=mybir.AluOpType.is_ge,
                fill=-1.0e5,
                base=0,
                pattern=[[1, P]],
                channel_multiplier=-1,
            )

            wqk = singles.tile([P, NCB, P], BF16, tag="wqk")
            nc.sync.dma_start(
                out=wqk, in_=wqk_in.rearrange("(cb p) m -> p cb m", p=P)
            )
            wv = singles.tile([P, NCB, HS], BF16, tag="wv")
            nc.sync.dma_start(
                out=wv, in_=wv_in.rearrange("(cb p) h -> p cb h", p=P)
            )

            # PE warm-up: ~12 back-to-back matmuls on the (tiny) weight
            # tile while x streams in.  Sustained PE activity releases the
            # HAM clock gate so the real matmuls start at full clock.
            wqk_flat = wqk.rearrange("p cb m -> p (cb m)")
            for _ in range(12):
                ps_w = p_psO.tile([HS, QG], F32, tag="o")
                nc.tensor.matmul(
                    ps_w, wqk[:, 0, 0:HS], wqk_flat[:, 0:QG],
                    start=True, stop=True,
                )

            bseq = [bb for _ in range(reps) for bb in range(BPC)]

            def load_xt(b):
                xt = p_xt.tile([P, NCB, T], BF16, tag="xt")
                for g in range(NQG):
                    nc.sync.dma_start(
                        out=xt[:, :, g * QG:(g + 1) * QG],
                        in_=xt_in[b, :, g * QG:(g + 1) * QG].rearrange(
                            "(cb p) t -> p cb t", p=P
                        ),
                    )
                return xt

            xt_next = load_xt(bseq[0])
            for bi, b in enumerate(bseq):
                # ---------------- stage A: projections ---------------
                xt = xt_next

                qt = p_qkv.tile([HS, T], BF16, tag="qt")
                kt = p_qkv.tile([HS, T], BF16, tag="kt")
                vn = p_qkv.tile([P, NT * HS], BF16, tag="vn")
                for g in range(NQG):
                    mv = [xt[:, cb, g * QG:(g + 1) * QG] for cb in range(NCB)]
                    ps_qk = p_psP.tile([P, QG], F32, tag="proj")
                    for cb in range(NCB):
                        nc.tensor.matmul(
                            ps_qk, wqk[:, cb, :], mv[cb],
                            start=(cb == 0), stop=(cb == NCB - 1),
                        )
                    nc.vector.tensor_copy(qt[:, g * QG:(g + 1) * QG], ps_qk[0:HS, :])
                    nc.vector.tensor_copy(kt[:, g * QG:(g + 1) * QG], ps_qk[HS:P, :])

                    ps_v = p_psP.tile([P, QG], F32, tag="proj")
                    for cb in range(NCB):
                        nc.tensor.matmul(
                            ps_v[0:HS, :], wv[:, cb, :], mv[cb],
                            start=(cb == 0), stop=(cb == NCB - 1),
                        )
                    vt = p_small.tile([HS, QG], BF16, tag="vt")
                    nc.vector.tensor_copy(vt, ps_v[0:HS, :])
                    ps_vt = p_psT.tile([P, 4 * HS], BF16, tag="tr")
                    for tt in range(4):
                        nc.tensor.transpose(
                            ps_vt[:, tt * HS:(tt + 1) * HS],
                            vt[:, tt * P:(tt + 1) * P],
                            ident,
                        )
                    nc.vector.tensor_copy(
                        vn[:, g * 4 * HS:(g + 1) * 4 * HS], ps_vt
                    )

                # prefetch next example's x while B/D of this one runs
                if bi + 1 < len(bseq):
                    xt_next = load_xt(bseq[bi + 1])

                # -------- stage B/D: scores, exp, sums, output --------
                pt = p_pt.tile([P, PTW], BF16, tag="pt")
                ctile = p_stats.tile([P, NT, 2], F32, tag="ctile")
                cinv = p_stats.tile([P, NT], F32, tag="cinv")
                vnr = p_qkv.tile([P, NT * HS], BF16, tag="vnr")

                def emit_out_chunk(qg):
                    ps_o = p_psO.tile([HS, QG], F32, tag="o")
                    nkb = 4 * (qg + 1)
                    for kb2 in range(nkb):
                        k0 = kb2 * P
                        qs = max(k0, qg * QG)
                        w = (qg + 1) * QG - qs
                        nc.tensor.matmul(
                            ps_o[:, qs - qg * QG:qs - qg * QG + w],
                            vnr[:, kb2 * HS:(kb2 + 1) * HS],
                            pt[:, PTOFF[kb2] + qs - k0:
                               PTOFF[kb2] + qs - k0 + w],
                            start=(kb2 == 0), stop=(kb2 == nkb - 1),
                        )
                    ot = p_small.tile([HS, QG], BF16, tag="ot")
                    nc.vector.tensor_copy(ot, ps_o)
                    ps_on = p_psT.tile([P, 4 * HS], BF16, tag="tr")
                    for tt in range(4):
                        nc.tensor.transpose(
                            ps_on[:, tt * HS:(tt + 1) * HS],
                            ot[:, tt * P:(tt + 1) * P],
                            ident,
                        )
                    on = p_small.tile([P, 4, HS], F32, tag="on")
                    nc.vector.tensor_copy(on, ps_on)
                    nc.sync.dma_start(
                        out=y_out[b, qg * QG:(qg + 1) * QG, :].rearrange(
                            "(tt p) h -> p tt h", p=P
                        ),
                        in_=on,
                    )

                for kb in range(NT):
                    # one row-group behind, so the stats chain for rows
                    # 4qg..4qg+3 hides behind row 4qg+4's score matmuls
                    if kb % 4 == 0 and kb > 0:
                        emit_out_chunk(kb // 4 - 1)
                    q0 = kb * P
                    off = PTOFF[kb]
                    L = T - q0
                    pieces = [(0, min(L, 2 * QG))]
                    if L > 2 * QG:
                        pieces.append((2 * QG, L - 2 * QG))
                    for ci, (ps, w) in enumerate(pieces):
                        ps_s = p_psS.tile([P, 2 * QG], F32, tag="s")
                        for ms in range(0, w, QG):
                            mw = min(QG, w - ms)
                            nc.tensor.matmul(
                                ps_s[:, ms:ms + mw],
                                kt[:, q0:q0 + P],
                                qt[:, q0 + ps + ms:q0 + ps + ms + mw],
                                start=True, stop=True,
                            )
                        if ci == 0:
                            nc.vector.tensor_add(
                                ps_s[:, 0:P], ps_s[:, 0:P], dmask
                            )
                        nc.scalar.activation(
                            out=pt[:, off + ps:off + ps + w],
                            in_=ps_s[:, :w],
                            func=mybir.ActivationFunctionType.Exp,
                            scale=SCL,
                            accum_out=ctile[:, kb, ci:ci + 1],
                        )

                    # per-row normalization stats + scaled V rows
                    if len(pieces) == 2:
                        crow = p_stats.tile([P, 1], F32, tag="crow")
                        nc.vector.reduce_sum(
                            crow, ctile[:, kb, :], axis=mybir.AxisListType.X
                        )
                        nc.vector.reciprocal(cinv[:, kb:kb + 1], crow)
                    else:
                        nc.vector.reciprocal(
                            cinv[:, kb:kb + 1], ctile[:, kb, 0:1]
                        )
                    nc.vector.tensor_scalar_mul(
                        vnr[:, kb * HS:(kb + 1) * HS],
                        vn[:, kb * HS:(kb + 1) * HS],
                        cinv[:, kb:kb + 1],
                    )
                emit_out_chunk(NQG - 1)
    _split_sync_waits(nc)
    return nc


_NC_CACHE = {}


def kernel(x, Wk, Wq, Wv, _reps=1):
    """Full-input entry point: shards over batch across 8 NeuronCores."""
    x = np.asarray(x, dtype=np.float32)
    Wk = np.asarray(Wk, dtype=np.float32)
    Wq = np.asarray(Wq, dtype=np.float32)
    Wv = np.asarray(Wv, dtype=np.float32)
    assert x.shape == (B, T, C), x.shape

    bf = ml_dtypes.bfloat16
    # host-side marshalling: c-major x, packed [Wq|Wk], all bf16
    xt = np.ascontiguousarray(x.transpose(0, 2, 1)).astype(bf)  # [B, C, T]
    wqk = np.ascontiguousarray(
        np.concatenate([Wq, Wk], axis=1)
    ).astype(bf)                                                # [C, 128]
    wv = np.ascontiguousarray(Wv).astype(bf)                    # [C, 64]

    if _reps not in _NC_CACHE:
        _NC_CACHE[_reps] = _build_nc(_reps)
    nc = _NC_CACHE[_reps]

    in_maps = [
        {
            "xt": xt[i * BPC:(i + 1) * BPC],
            "wqk": wqk,
            "wv": wv,
        }
        for i in range(NCORES)
    ]
    res = run_bass_kernel_spmd(nc, in_maps, list(range(NCORES)))
    return np.concatenate(
        [res.results[i]["out"] for i in range(NCORES)], axis=0
    )
